# revision 20
# baseline (speedup 1.0000x reference)
"""GAT+LSTM Trainium2 kernel: 8-core SPMD, batch data-parallel (512 rows/core).

GAT: each core computes GAT outputs only for the unique nodes its batch slice
references (edges sharded by dst, sorted, grouped into 16-dst windows;
self-loop attrs = host-computed means). Logits via matmul against gathered
node features; scatter-softmax via one-hot masked matmuls.

LSTM: all-sigmoid formulation (tanh(x) = 2*sigmoid(2x)-1 folded into weight/
state scaling), layer 1 lagged one step behind layer 0 so both layers' gate
sigmoids pipeline; 3 ACT instructions per step; fused scalar_tensor_tensor
elementwise; rank-4 matmul for layer-1 bias.
"""
import os
import sys

sys.path.insert(0, "/opt/trn_rl_repo")

import numpy as np
import ml_dtypes

import concourse.bass as bass
import concourse.tile as tile
from concourse import bacc, mybir
from concourse import bass_utils

F32 = mybir.dt.float32
BF16 = mybir.dt.bfloat16
FP16 = mybir.dt.float16
AF = mybir.ActivationFunctionType
ALU = mybir.AluOpType

N_CORES = 8
N_NODES = 20000
BATCH = 4096
BC = BATCH // N_CORES      # 512
SEQ_LEN = 50
SEQ_F = 32
NODE_F = 128
HEADS = 4
GAT_OUT = 64
LSTM_H = 128
SPAN = 2048

# torch gate order i,f,g,o -> ours i,f,o,g
GPERM = np.r_[0:128, 128:256, 384:512, 256:384]
G_SLOT = slice(384, 512)   # g-gate rows in permuted [512] gate layout


def _f(x):
    return np.asarray(x, np.float32)


def host_prep(inputs):
    ei = np.asarray(inputs['edge_index'])
    ea = _f(inputs['edge_attr'])
    nidx = np.asarray(inputs['node_indices'])
    seqs = _f(inputs['sequences'])
    x = _f(inputs['node_features'])
    xb = x.astype(ml_dtypes.bfloat16)

    # ---- folded GAT weights ----
    Vs_l, Vd_l, w_e4_l, b4_l = [], [], [], []
    Wstk = np.zeros((128, 8, GAT_OUT), np.float32)
    gb = np.zeros((GAT_OUT, 2), np.float32)
    for li in (1, 2):
        lw_h = _f(inputs[f'g{li}_lin_w']).reshape(128, HEADS, GAT_OUT)
        a_s = _f(inputs[f'g{li}_att_src'])
        a_d = _f(inputs[f'g{li}_att_dst'])
        a_e = _f(inputs[f'g{li}_att_edge'])
        lew = _f(inputs[f'g{li}_lin_edge_w']).reshape(GAT_OUT, HEADS, GAT_OUT)
        Vs_l.append(np.einsum('dhc,hc->dh', lw_h, a_s))
        Vd_l.append(np.einsum('dhc,hc->dh', lw_h, a_d))
        ve = np.einsum('dhc,hc->dh', lew, a_e)
        w_e4_l.append(_f(inputs['eat_w']) @ ve)
        b4_l.append(_f(inputs['eat_b']) @ ve)
        Wstk[:, (li - 1) * 4:(li - 1) * 4 + 4, :] = lw_h / HEADS
        gb[:, li - 1] = _f(inputs[f'g{li}_bias'])
    Vsrc = np.concatenate(Vs_l, 1)          # [128,8]
    Vdst = np.concatenate(Vd_l, 1)
    w_e4 = np.concatenate(w_e4_l, 1)        # [16,8]
    b4 = np.concatenate(b4_l, 0)            # [8]
    z16 = np.zeros((128, 16), np.float32)
    Vs_dup = np.ascontiguousarray(np.concatenate([Vsrc, Vsrc, z16], 1)).astype(ml_dtypes.bfloat16)
    Vd_dup = np.ascontiguousarray(np.concatenate([Vdst, Vdst, z16], 1)).astype(ml_dtypes.bfloat16)
    w18 = np.zeros((18, 32), np.float32)
    w18[:16, :16] = np.concatenate([w_e4, w_e4], 1)
    w18[16, :16] = np.concatenate([b4, b4])
    w18[17, :16] = -40.0
    w18 = w18.astype(ml_dtypes.bfloat16)

    src = ei[0].astype(np.int64)
    dst = ei[1].astype(np.int64)
    # host-computed self-loop attrs: mean of incoming raw edge_attr per node
    sums = np.zeros((N_NODES, 16), np.float32)
    np.add.at(sums, dst, ea)
    cnt_all = np.bincount(dst, minlength=N_NODES).astype(np.float32)
    loop_mean = sums / np.maximum(cnt_all, 1.0)[:, None]

    # ---- LSTM weights: permute gates to i,f,o,g; fold tanh->sigmoid (x2 on
    # g rows) and h = 2*hhat (x2 on h-input columns) ----
    w_ih0 = _f(inputs['w_ih0'])[GPERM]
    w_hh0 = _f(inputs['w_hh0'])[GPERM]
    b0 = (_f(inputs['b_ih0']) + _f(inputs['b_hh0']))[GPERM]
    w_ih1 = _f(inputs['w_ih1'])[GPERM]
    w_hh1 = _f(inputs['w_hh1'])[GPERM]
    b1 = (_f(inputs['b_ih1']) + _f(inputs['b_hh1']))[GPERM]
    for W, b in ((w_ih0, b0), (w_hh0, None), (w_ih1, b1), (w_hh1, None)):
        W[G_SLOT.start:G_SLOT.stop] *= 2.0
        if b is not None:
            b[G_SLOT.start:G_SLOT.stop] *= 2.0
    # x2 on recurrent-h input columns (h = 2*hhat)
    w_hh0 *= 2.0
    w_ih1 *= 2.0
    w_hh1 *= 2.0
    fcw2 = 2.0 * _f(inputs['fc_w']).reshape(128, 1)

    WseqT = np.zeros((SEQ_F + 1, 512), np.float32)
    WseqT[:SEQ_F] = w_ih0[:, :SEQ_F].T
    WseqT[SEQ_F] = b0
    shared = dict(
        Vs_dup=Vs_dup, Vd_dup=Vd_dup, w18=w18,
        Wstk=np.ascontiguousarray(Wstk.astype(ml_dtypes.bfloat16)), gb=gb,
        I128b=np.eye(128, dtype=ml_dtypes.bfloat16),
        iota16rep=np.ascontiguousarray(
            np.tile(np.arange(16, dtype=np.float32), (128, 8))).astype(ml_dtypes.bfloat16),
        ones_bf=np.ones((128, 1), ml_dtypes.bfloat16),
        WseqT=np.ascontiguousarray(WseqT).astype(ml_dtypes.bfloat16),
        Wg=np.ascontiguousarray(w_ih0[:, SEQ_F:].T).astype(ml_dtypes.bfloat16),
        Whh0=np.ascontiguousarray(w_hh0.T).astype(ml_dtypes.bfloat16),
        Wih1=np.ascontiguousarray(w_ih1.T).astype(ml_dtypes.bfloat16),
        Whh1=np.ascontiguousarray(w_hh1.T).astype(ml_dtypes.bfloat16),
        b1lhs=np.ascontiguousarray(b1.reshape(1, 4, 128)).astype(ml_dtypes.bfloat16),
        ones_row=np.ones((1, 512), ml_dtypes.bfloat16),
        fcw2=fcw2.astype(ml_dtypes.bfloat16),
        fcb=float(_f(inputs['fc_b']).reshape(-1)[0]),
    )

    cores = []
    for c in range(N_CORES):
        sel = nidx[c * BC:(c + 1) * BC].astype(np.int64)
        uniq = np.unique(sel)
        U = len(uniq)
        n_win = (U + 15) // 16
        U_pad = n_win * 16
        kd_pos = np.searchsorted(uniq, dst)
        keep = (kd_pos < U) & (uniq[np.minimum(kd_pos, U - 1)] == dst)
        ks = src[keep]
        ku = np.searchsorted(uniq, dst[keep])
        kea = ea[keep]
        order = np.argsort(ku, kind='stable')
        ks, ku, kea = ks[order], ku[order], kea[order]
        ubnd = np.searchsorted(ku, np.arange(0, U_pad + 16, 16))

        e_src, e_cdst, e_ea, e_kind = [], [], [], []
        win_off, win_nch = [], []
        for w in range(n_win):
            off = sum(len(a) for a in e_src)
            win_off.append(off)
            u0 = w * 16
            nreal_u = min(16, U - u0)
            ss = np.zeros(16, np.int64)
            ss[:nreal_u] = uniq[u0:u0 + nreal_u]
            e_src.append(ss)
            e_cdst.append(np.arange(16, dtype=np.float32))
            sea = np.zeros((16, 16), np.float32)
            sea[:nreal_u] = loop_mean[uniq[u0:u0 + nreal_u]]
            e_ea.append(sea)
            kk = np.full(16, 1, np.int64)
            kk[nreal_u:] = 2
            e_kind.append(kk)
            lo, hi = ubnd[w], ubnd[w + 1]
            nreal = hi - lo
            e_src.append(ks[lo:hi])
            e_cdst.append((ku[lo:hi] - u0).astype(np.float32))
            e_ea.append(kea[lo:hi])
            e_kind.append(np.zeros(nreal, np.int64))
            npad = (-(16 + nreal)) % 128
            if npad:
                e_src.append(np.zeros(npad, np.int64))
                e_cdst.append(np.zeros(npad, np.float32))
                e_ea.append(np.zeros((npad, 16), np.float32))
                e_kind.append(np.full(npad, 2, np.int64))
            win_nch.append((16 + nreal + npad) // 128)
        e_src = np.concatenate(e_src)
        e_cdst = np.concatenate(e_cdst)
        e_ea = np.concatenate(e_ea)
        e_kind = np.concatenate(e_kind)
        sq = seqs[c * BC:(c + 1) * BC]
        seqT = np.ones((SEQ_LEN, SEQ_F + 1, BC), np.float32)
        seqT[:, :SEQ_F, :] = sq.transpose(1, 2, 0)
        cores.append(dict(
            n_win=n_win, U=U, U_pad=U_pad, win_off=win_off, win_nch=win_nch,
            e_src=e_src, e_cdst=e_cdst, e_ea=e_ea, e_kind=e_kind,
            map_b=np.searchsorted(uniq, sel).astype(np.int64),
            seqT=np.ascontiguousarray(seqT).astype(ml_dtypes.bfloat16),
        ))

    # ---- uniform padding across cores: same n_win AND same chunks/window ----
    nwmax = max(co['n_win'] for co in cores)
    nchw = max(max(co['win_nch']) for co in cores)
    padW = nchw * 128
    for co in cores:
        ns, ncd, nea, nk = [], [], [], []
        new_off, new_nch = [], []
        for w in range(nwmax):
            new_off.append(w * padW)
            new_nch.append(nchw)
            if w < co['n_win']:
                off = co['win_off'][w]
                n = co['win_nch'][w] * 128
                ns.append(co['e_src'][off:off + n])
                ncd.append(co['e_cdst'][off:off + n])
                nea.append(co['e_ea'][off:off + n])
                nk.append(co['e_kind'][off:off + n])
                pad = padW - n
            else:
                pad = padW
            if pad:
                ns.append(np.zeros(pad, np.int64))
                ncd.append((np.arange(pad) % 16).astype(np.float32))
                nea.append(np.zeros((pad, 16), np.float32))
                nk.append(np.full(pad, 2, np.int64))
        co['e_src'] = np.concatenate(ns)
        co['e_cdst'] = np.concatenate(ncd)
        co['e_ea'] = np.concatenate(nea)
        co['e_kind'] = np.concatenate(nk)
        co['win_off'], co['win_nch'], co['n_win'] = new_off, new_nch, nwmax
    Emax = ((nwmax * padW + SPAN - 1) // SPAN) * SPAN
    for co in cores:
        add = Emax - len(co['e_src'])
        if add:
            co['e_src'] = np.concatenate([co['e_src'], np.zeros(add, np.int64)])
            co['e_cdst'] = np.concatenate([co['e_cdst'],
                                           (np.arange(add) % 16).astype(np.float32)])
            co['e_ea'] = np.concatenate([co['e_ea'], np.zeros((add, 16), np.float32)])
            co['e_kind'] = np.concatenate([co['e_kind'], np.full(add, 2, np.int64)])
        E, kind = Emax, co['e_kind']
        nch = E // 128
        eaT = np.zeros((18, E), np.float32)
        eaT[:16] = co['e_ea'].T
        eaT[16] = (kind != 2)
        eaT[17] = (kind == 2)
        # dst node ids for the d-gather
        dstn = np.zeros(E, np.int64)
        for w in range(co['n_win']):
            off = co['win_off'][w]
            n = co['win_nch'][w] * 128
            u_ids = co['e_src'][off:off + 16]
            j = np.minimum(co['e_cdst'][off:off + n].astype(np.int64), 15)
            dstn[off:off + n] = u_ids[j]
        co.update(
            E=E, nch=nch,
            eaT=np.ascontiguousarray(eaT).astype(ml_dtypes.bfloat16),
            Xg_h=np.ascontiguousarray(
                xb[co['e_src']].reshape(nch, 128, 128).transpose(1, 0, 2)),
            XsT_h=np.ascontiguousarray(xb[co['e_src']].T),
            XdT_h=np.ascontiguousarray(xb[dstn].T),
            cdst16=np.ascontiguousarray(co['e_cdst'].reshape(nch, 128).T),
            U_pad=nwmax * 16,
        )
        Sel = np.zeros((nwmax * 16, BC), np.float32)
        Sel[co['map_b'], np.arange(BC)] = 1.0
        co['Sel'] = np.ascontiguousarray(
            Sel.reshape(nwmax * 16 // 128, 128, BC).transpose(1, 0, 2)).astype(ml_dtypes.bfloat16)
    return cores, shared


def build_core_program(nc, co):
    E, nch, n_win, U_pad = co['E'], co['nch'], co['n_win'], co['U_pad']
    n_span = E // SPAN

    def din(name, shape, dt):
        return nc.dram_tensor(name, list(shape), dt, kind="ExternalInput")

    seqT_d = din('seqT', (SEQ_LEN, SEQ_F + 1, BC), BF16)
    Xg_d = din('Xg_h', (128, nch, 128), BF16)
    XsT_d = din('XsT_h', (128, E), BF16)
    XdT_d = din('XdT_h', (128, E), BF16)
    Sel_d = din('Sel', (128, U_pad // 128, BC), BF16)
    eaT_d = din('eaT', (18, E), BF16)
    cdst16_d = din('cdst16', (128, nch), F32)
    Vs_d = din('Vs_dup', (128, 32), BF16)
    Vd_d = din('Vd_dup', (128, 32), BF16)
    w18_d = din('w18', (18, 32), BF16)
    Wstk_d = din('Wstk', (128, 8, GAT_OUT), BF16)
    gb_d = din('gb', (GAT_OUT, 2), F32)
    I128b_d = din('I128b', (128, 128), BF16)
    iota_d = din('iota16rep', (128, 128), BF16)
    ones_d = din('ones_bf', (128, 1), BF16)
    WseqT_d = din('WseqT', (SEQ_F + 1, 512), BF16)
    Wg_d = din('Wg', (128, 512), BF16)
    Whh0_d = din('Whh0', (128, 512), BF16)
    Wih1_d = din('Wih1', (128, 512), BF16)
    Whh1_d = din('Whh1', (128, 512), BF16)
    b1lhs_d = din('b1lhs', (1, 4, 128), BF16)
    ones_row_d = din('ones_row', (1, 512), BF16)
    fcw2_d = din('fcw2', (128, 1), BF16)
    fcb_d = din('fcb', (1, 1), F32)
    y_d = nc.dram_tensor('y', [1, BC], F32, kind="ExternalOutput")

    # window/chunk bookkeeping (host-known)
    chunk_win = []
    for w in range(n_win):
        chunk_win += [w] * co['win_nch'][w]
    chunk_win += [-1] * (nch - len(chunk_win))
    win_first_last = {}
    for c, w in enumerate(chunk_win):
        if w < 0:
            continue
        if w not in win_first_last:
            win_first_last[w] = [c, c]
        win_first_last[w][1] = c
    WGRP = 4   # windows finalized per group

    import contextlib
    with tile.TileContext(nc) as tc:
        with contextlib.ExitStack() as ctx:
            consts = ctx.enter_context(tc.tile_pool(name="consts", bufs=1))

            def load(dram, shape, dt):
                nm = dram.ap().tensor.name
                t = consts.tile(list(shape), dt, name="c_" + nm, tag="c_" + nm)
                nc.sync.dma_start(t[:], dram.ap())
                return t

            I128b = load(I128b_d, (128, 128), BF16)
            iota = load(iota_d, (128, 128), BF16)
            ones = load(ones_d, (128, 1), BF16)
            Vs = load(Vs_d, (128, 32), BF16)
            Vd = load(Vd_d, (128, 32), BF16)
            w18 = load(w18_d, (18, 32), BF16)
            Wstk = load(Wstk_d, (128, 8, GAT_OUT), BF16)
            gbias = load(gb_d, (GAT_OUT, 2), F32)
            cdst = load(cdst16_d, (128, nch), F32)
            Sel = load(Sel_d, (128, U_pad // 128, BC), BF16)
            Wseq = load(WseqT_d, (SEQ_F + 1, 512), BF16)
            Wg = load(Wg_d, (128, 512), BF16)
            Whh0 = load(Whh0_d, (128, 512), BF16)
            Wih1 = load(Wih1_d, (128, 512), BF16)
            Whh1 = load(Whh1_d, (128, 512), BF16)
            b1lhs = load(b1lhs_d, (1, 4, 128), BF16)
            ones_row = load(ones_row_d, (1, 512), BF16)
            fcw2 = load(fcw2_d, (128, 1), BF16)
            fcb = load(fcb_d, (1, 1), F32)

            persist = ctx.enter_context(tc.tile_pool(name="persist", bufs=1))
            T_sb = persist.tile([128, n_span * 512], BF16)     # transposed exp(leaky) blocks
            AnT_all = persist.tile([128, n_win, 128], BF16)
            gcombT = persist.tile([128, BC], BF16)

            # ================= GAT =================
            with contextlib.ExitStack() as gctx:
                span_pool = gctx.enter_context(tc.tile_pool(name="span", bufs=3))
                sd_ps = gctx.enter_context(tc.tile_pool(name="sd_ps", bufs=2, space="PSUM"))
                g_pool = gctx.enter_context(tc.tile_pool(name="g", bufs=2))
                tp_ps = gctx.enter_context(tc.tile_pool(name="tp_ps", bufs=2, space="PSUM"))
                pall_pool = gctx.enter_context(tc.tile_pool(name="pall", bufs=4))
                agg_ps = gctx.enter_context(tc.tile_pool(name="agg_ps", bufs=2, space="PSUM"))
                den_ps = gctx.enter_context(tc.tile_pool(name="den_ps", bufs=2, space="PSUM"))
                fin_pool = gctx.enter_context(tc.tile_pool(name="fin", bufs=2))

                agg_group = {}
                for sp in range(n_span):
                    sc0 = sp * SPAN
                    eaT_sp = span_pool.tile([18, SPAN], BF16, tag="easp")
                    nc.sync.dma_start(eaT_sp[:], eaT_d.ap()[:, sc0:sc0 + SPAN])
                    Xg = span_pool.tile([128, SPAN // 128, 128], BF16, tag="xg")
                    nc.sync.dma_start(Xg[:], Xg_d.ap()[:, sc0 // 128:(sc0 + SPAN) // 128, :])
                    XsT = span_pool.tile([128, SPAN], BF16, tag="xst")
                    nc.sync.dma_start(XsT[:], XsT_d.ap()[:, sc0:sc0 + SPAN])
                    XdT = span_pool.tile([128, SPAN], BF16, tag="xdt")
                    nc.sync.dma_start(XdT[:], XdT_d.ap()[:, sc0:sc0 + SPAN])

                    # --- logits: S[32k+0:32k+32, :] for 4 k-chunks of 512 edges ---
                    S_ps = sd_ps.tile([128, 512], F32, tag="S")
                    for k in range(4):
                        cl = 512 * k
                        nc.tensor.matmul(S_ps[32 * k:32 * k + 32, :], lhsT=Vs[:],
                                         rhs=XsT[:, cl:cl + 512], start=True, stop=False,
                                         tile_position=(0, 32 * k))
                        nc.tensor.matmul(S_ps[32 * k:32 * k + 32, :], lhsT=Vd[:],
                                         rhs=XdT[:, cl:cl + 512], start=False, stop=False,
                                         tile_position=(0, 32 * k))
                        nc.tensor.matmul(S_ps[32 * k:32 * k + 32, :], lhsT=w18[:],
                                         rhs=eaT_sp[:, cl:cl + 512], start=False, stop=True,
                                         tile_position=(0, 32 * k))
                    # PSUM->SBUF copy on ACT (idle in GAT), then leaky on DVE
                    G1 = g_pool.tile([128, 512], BF16, tag="G1")
                    nc.scalar.copy(G1[:], S_ps[:])
                    G = g_pool.tile([128, 512], BF16, tag="G")
                    nc.vector.scalar_tensor_tensor(G[:], G1[:], 0.2, G1[:],
                                                   op0=ALU.mult, op1=ALU.max)
                    # transpose 4 blocks -> T region, then exp in-place
                    tps = tp_ps.tile([128, 512], BF16, tag="tps")
                    for jb in range(4):
                        nc.tensor.transpose(tps[:, 128 * jb:128 * jb + 128],
                                            G[:, 128 * jb:128 * jb + 128], I128b[:])
                    tcol0 = sp * 512
                    nc.scalar.activation(T_sb[:, tcol0:tcol0 + 512], tps[:], AF.Exp)

                    # --- aggregation for chunks in this span ---
                    for c in range(sc0 // 128, (sc0 + SPAN) // 128):
                        w = chunk_win[c]
                        if w < 0:
                            continue
                        grp = w // WGRP
                        wi = w % WGRP
                        c_first, c_last = win_first_last[w]
                        if wi == 0 and c == c_first:
                            agg_group[grp] = (
                                agg_ps.tile([128, WGRP, 128], F32, tag="aggp", name="aggp"),
                                den_ps.tile([128, WGRP], F32, tag="aggd", name="aggd"))
                        aggp, aggd = agg_group[grp]
                        e0 = c * 128
                        k = (e0 - sc0) // 512
                        jb = ((e0 - sc0) % 512) // 128
                        tcol = sp * 512 + 128 * jb + 32 * k
                        pall = pall_pool.tile([128, 128], BF16, tag="pall")
                        eng = nc.vector
                        eng.scalar_tensor_tensor(
                            pall[:].rearrange("p (h u) -> p h u", h=8),
                            iota[:].rearrange("p (h u) -> p h u", h=8),
                            cdst[:, c:c + 1],
                            T_sb[:, tcol:tcol + 8].unsqueeze(2).broadcast_to([128, 8, 16]),
                            op0=ALU.is_equal, op1=ALU.mult)
                        nc.tensor.matmul(aggp[:, wi, :], lhsT=pall[:],
                                         rhs=Xg[:, (e0 - sc0) // 128, :],
                                         start=(c == c_first), stop=(c == c_last))
                        nc.tensor.matmul(aggd[:, wi:wi + 1], lhsT=pall[:], rhs=ones[:],
                                         start=(c == c_first), stop=(c == c_last))
                        if c == c_last and wi == WGRP - 1:
                            rec = fin_pool.tile([128, WGRP], F32, tag="rec")
                            nc.vector.reciprocal(rec[:], aggd[:])
                            anw = fin_pool.tile([128, WGRP, 128], BF16, tag="anw")
                            nc.vector.tensor_tensor(
                                anw[:], aggp[:],
                                rec[:].unsqueeze(2).broadcast_to([128, WGRP, 128]),
                                op=ALU.mult)
                            antp = tp_ps.tile([128, 512], BF16, tag="tps", name="antp")
                            for wj in range(WGRP):
                                nc.tensor.transpose(antp[:, 128 * wj:128 * wj + 128],
                                                    anw[:, wj, :], I128b[:])
                            w0 = grp * WGRP
                            nc.vector.tensor_copy(
                                AnT_all[:, w0:w0 + WGRP, :].rearrange("p a b -> p (a b)"),
                                antp[:])
                            del agg_group[grp]

                # --- projection + gcomb ---
                o1 = agg_ps.tile([GAT_OUT, U_pad], F32, tag="aggp", name="o1")
                o2 = agg_ps.tile([GAT_OUT, U_pad], F32, tag="aggp", name="o2")
                for h in range(4):
                    nc.tensor.matmul(o1[:], lhsT=Wstk[:, h, :],
                                     rhs=AnT_all[:, :, 16 * h:16 * h + 16],
                                     start=(h == 0), stop=(h == 3))
                    nc.tensor.matmul(o2[:], lhsT=Wstk[:, 4 + h, :],
                                     rhs=AnT_all[:, :, 64 + 16 * h:64 + 16 * h + 16],
                                     start=(h == 0), stop=(h == 3))
                gstk = fin_pool.tile([128, U_pad], BF16, tag="gstk")
                nc.scalar.add(gstk[0:64, :], o1[:], gbias[:, 0:1])
                nc.scalar.add(gstk[64:128, :], o2[:], gbias[:, 1:2])
                gsel = agg_ps.tile([128, BC], F32, tag="aggp", name="gsel")
                for uc in range(U_pad // 128):
                    gtp = tp_ps.tile([128, 128], BF16, tag="tps", name="gtp")
                    nc.tensor.transpose(gtp[:], gstk[:, 128 * uc:128 * uc + 128], I128b[:])
                    gts = fin_pool.tile([128, 128], BF16, tag="gts")
                    nc.vector.tensor_copy(gts[:], gtp[:])
                    nc.tensor.matmul(gsel[:], lhsT=gts[:], rhs=Sel[:, uc, :],
                                     start=(uc == 0), stop=(uc == U_pad // 128 - 1))
                nc.vector.tensor_copy(gcombT[:], gsel[:])

            # ================= LSTM =================
            seq_pool = ctx.enter_context(tc.tile_pool(name="seq", bufs=2))
            ps0_pool = ctx.enter_context(tc.tile_pool(name="ps0", bufs=1, space="PSUM"))
            ps1_pool = ctx.enter_context(tc.tile_pool(name="ps1", bufs=1, space="PSUM"))
            st_pool = ctx.enter_context(tc.tile_pool(name="state", bufs=1))
            act_pool = ctx.enter_context(tc.tile_pool(name="acts", bufs=2))

            psum0 = ps0_pool.tile([128, 2048], F32)
            psum1 = ps1_pool.tile([128, 2048], F32)
            h0 = [st_pool.tile([128, BC], BF16, tag=f"h0{p}", name=f"h0{p}")
                  for p in (0, 1)]
            h1 = [st_pool.tile([128, BC], BF16, tag=f"h1{p}", name=f"h1{p}")
                  for p in (0, 1)]
            ctil = [st_pool.tile([128, 2, BC], FP16, tag=f"ct{p}", name=f"ct{p}")
                    for p in (0, 1)]
            for t_ in h0 + h1 + ctil:
                nc.vector.memset(t_[:], 0.0)

            TBLK = 10
            seqb = None
            for t in range(SEQ_LEN + 1):
                par = t % 2
                op = 1 - par
                if t < SEQ_LEN:
                    if t % TBLK == 0:
                        seqb = seq_pool.tile([SEQ_F + 1, TBLK, BC], BF16)
                        nc.sync.dma_start(
                            seqb[:], seqT_d.ap()[t:t + TBLK, :, :].rearrange("t p b -> p t b"))
                    tt = t % TBLK
                    # ---- L0 gates(t): seq + gcomb + rec ----
                    for g in range(4):
                        o = psum0[:, 512 * g:512 * g + 512]
                        nc.tensor.matmul(o, lhsT=Wseq[:, 128 * g:128 * g + 128],
                                         rhs=seqb[:, tt, :], start=True, stop=False)
                        nc.tensor.matmul(o, lhsT=Wg[:, 128 * g:128 * g + 128],
                                         rhs=gcombT[:], start=False, stop=False)
                        nc.tensor.matmul(o, lhsT=Whh0[:, 128 * g:128 * g + 128],
                                         rhs=h0[op][:], start=False, stop=True)
                    sig0 = act_pool.tile([128, 2048], FP16, tag="sig0")
                    nc.scalar.activation(sig0[:], psum0[:], AF.Sigmoid)
                    # ---- L0 elementwise: c0(t) ----
                    t2_0 = act_pool.tile([128, BC], FP16, tag="t20")
                    nc.vector.tensor_tensor(t2_0[:], sig0[:, 512:1024],
                                            ctil[op][:, 0, :], op=ALU.mult)
                    t1_0 = act_pool.tile([128, BC], FP16, tag="t10")
                    nc.vector.scalar_tensor_tensor(t1_0[:], sig0[:, 1536:2048], -0.5,
                                                   sig0[:, 0:512],
                                                   op0=ALU.add, op1=ALU.mult)
                    nc.vector.scalar_tensor_tensor(ctil[par][:, 0, :], t1_0[:], 4.0,
                                                   t2_0[:], op0=ALU.mult, op1=ALU.add)
                # ---- L1 gates(t-1) ----
                sig1 = act_pool.tile([128, 2048], FP16, tag="sig1")
                for g in range(4):
                    o = psum1[:, 512 * g:512 * g + 512]
                    nc.tensor.matmul(o, lhsT=b1lhs[:, g, :], rhs=ones_row[:],
                                     start=True, stop=False)
                    nc.tensor.matmul(o, lhsT=Wih1[:, 128 * g:128 * g + 128],
                                     rhs=h0[op][:], start=False, stop=False)
                    nc.tensor.matmul(o, lhsT=Whh1[:, 128 * g:128 * g + 128],
                                     rhs=h1[op][:], start=False, stop=True)
                nc.scalar.activation(sig1[:], psum1[:], AF.Sigmoid)
                t2_1 = act_pool.tile([128, BC], FP16, tag="t21")
                nc.gpsimd.tensor_tensor(t2_1[:], sig1[:, 512:1024],
                                        ctil[op][:, 1, :], op=ALU.mult)
                t1_1 = act_pool.tile([128, BC], FP16, tag="t11")
                nc.vector.scalar_tensor_tensor(t1_1[:], sig1[:, 1536:2048], -0.5,
                                               sig1[:, 0:512], op0=ALU.add, op1=ALU.mult)
                nc.vector.scalar_tensor_tensor(ctil[par][:, 1, :], t1_1[:], 4.0,
                                               t2_1[:], op0=ALU.mult, op1=ALU.add)
                # ---- sigma(ctil) both layers, then h-hats ----
                sigc = act_pool.tile([128, 2, BC], FP16, tag="sigc")
                if t < SEQ_LEN:
                    nc.scalar.activation(sigc[:], ctil[par][:], AF.Sigmoid)
                    nc.vector.scalar_tensor_tensor(h0[par][:], sigc[:, 0, :], -0.5,
                                                   sig0[:, 1024:1536],
                                                   op0=ALU.add, op1=ALU.mult)
                else:
                    nc.scalar.activation(sigc[:, 1, :], ctil[par][:, 1, :], AF.Sigmoid)
                nc.vector.scalar_tensor_tensor(h1[par][:], sigc[:, 1, :], -0.5,
                                               sig1[:, 1024:1536],
                                               op0=ALU.add, op1=ALU.mult)
                if t == 0:
                    # discard the fictitious L1 step: h1(-1) = c1(-1) = 0
                    nc.vector.memset(h1[0][:], 0.0)
                    nc.vector.memset(ctil[0][:, 1, :], 0.0)

            # ---------------- fc ----------------
            yps = psum0[0:1, 0:BC]
            nc.tensor.matmul(yps, lhsT=fcw2[:], rhs=h1[(SEQ_LEN) % 2][:],
                             start=True, stop=True)
            ysb = act_pool.tile([1, BC], F32, tag="ysb")
            nc.scalar.add(ysb[:], yps, fcb[:1, :1])
            nc.sync.dma_start(y_d.ap(), ysb[:])


def kernel(**inputs):
    cores, sh = host_prep(inputs)
    co0 = cores[0]

    nc = bacc.Bacc("TRN2", target_bir_lowering=False, debug=False, num_devices=1)
    build_core_program(nc, co0)
    nc.compile()

    in_maps = []
    for co in cores:
        m = dict(
            seqT=co['seqT'], Xg_h=co['Xg_h'], XsT_h=co['XsT_h'], XdT_h=co['XdT_h'],
            Sel=co['Sel'], eaT=co['eaT'], cdst16=co['cdst16'],
            fcb=np.array([[sh['fcb']]], np.float32),
        )
        for k in ('Vs_dup', 'Vd_dup', 'w18', 'Wstk', 'gb', 'I128b', 'iota16rep',
                  'ones_bf', 'WseqT', 'Wg', 'Whh0', 'Wih1', 'Whh1', 'b1lhs',
                  'ones_row', 'fcw2'):
            m[k] = sh[k]
        in_maps.append(m)

    if os.environ.get("BK_SIM"):
        from concourse.bass_interp import CoreSim
        ncore = int(os.environ.get("BK_SIM_CORES", "1"))
        outs = []
        for ci in range(ncore):
            sim = CoreSim(nc, require_finite=False, require_nnan=False)
            for k, v in in_maps[ci].items():
                sim.tensor(k)[:] = v
            sim.simulate(check_with_hw=False)
            outs.append(np.array(sim.tensor('y')).reshape(BC, 1).copy())
        for ci in range(ncore, N_CORES):
            outs.append(np.zeros((BC, 1), np.float32))
        return np.concatenate(outs, 0)

    trace = bool(os.environ.get("BK_TRACE"))
    res = bass_utils.run_bass_kernel_spmd(nc, in_maps, core_ids=list(range(N_CORES)),
                                          trace=trace)
    if trace:
        global LAST_EXEC_NS, LAST_RES
        LAST_EXEC_NS = res.exec_time_ns
        LAST_RES = res
        print("HW exec time:", res.exec_time_ns, "ns")
    return np.concatenate([res.results[c]['y'].reshape(BC, 1) for c in range(N_CORES)], 0)


LAST_EXEC_NS = None
LAST_RES = None


# revision 31
# speedup vs baseline: 1.3803x; 1.3803x over previous
"""GAT+LSTM Trainium2 kernel: 8-core SPMD, batch data-parallel (512 rows/core).

GAT: each core computes GAT outputs only for the unique nodes its batch slice
references (edges sharded by dst, sorted, grouped into 16-dst windows;
self-loop attrs = host-computed means). Logits via matmul against gathered
node features; scatter-softmax via one-hot masked matmuls.

LSTM: all-sigmoid formulation (tanh(x) = 2*sigmoid(2x)-1 folded into weight/
state scaling), layer 1 lagged one step behind layer 0 so both layers' gate
sigmoids pipeline; 3 ACT instructions per step; fused scalar_tensor_tensor
elementwise; rank-4 matmul for layer-1 bias.
"""
import os
import sys

sys.path.insert(0, "/opt/trn_rl_repo")

import numpy as np
import ml_dtypes

import concourse.bass as bass
import concourse.tile as tile
from concourse import bacc, mybir
from concourse import bass_utils

F32 = mybir.dt.float32
BF16 = mybir.dt.bfloat16
FP16 = mybir.dt.float16
AF = mybir.ActivationFunctionType
ALU = mybir.AluOpType

N_CORES = 8
N_NODES = 20000
BATCH = 4096
BC = BATCH // N_CORES      # 512
SEQ_LEN = 50
SEQ_F = 32
NODE_F = 128
HEADS = 4
GAT_OUT = 64
LSTM_H = 128
SPAN = 2048

# gate order stays torch's i,f,g,o (g-block contiguous with i,f for one sigmoid)
G_SLOT = slice(256, 384)   # g-gate rows in [512] gate layout


def _f(x):
    return np.asarray(x, np.float32)


def host_prep(inputs):
    ei = np.asarray(inputs['edge_index'])
    ea = _f(inputs['edge_attr'])
    nidx = np.asarray(inputs['node_indices'])
    seqs = _f(inputs['sequences'])
    x = _f(inputs['node_features'])
    xb = x.astype(ml_dtypes.bfloat16)

    # ---- folded GAT weights ----
    Vs_l, Vd_l, w_e4_l, b4_l = [], [], [], []
    Wstk = np.zeros((128, 8, GAT_OUT), np.float32)
    gb = np.zeros((GAT_OUT, 2), np.float32)
    for li in (1, 2):
        lw_h = _f(inputs[f'g{li}_lin_w']).reshape(128, HEADS, GAT_OUT)
        a_s = _f(inputs[f'g{li}_att_src'])
        a_d = _f(inputs[f'g{li}_att_dst'])
        a_e = _f(inputs[f'g{li}_att_edge'])
        lew = _f(inputs[f'g{li}_lin_edge_w']).reshape(GAT_OUT, HEADS, GAT_OUT)
        Vs_l.append(np.einsum('dhc,hc->dh', lw_h, a_s))
        Vd_l.append(np.einsum('dhc,hc->dh', lw_h, a_d))
        ve = np.einsum('dhc,hc->dh', lew, a_e)
        w_e4_l.append(_f(inputs['eat_w']) @ ve)
        b4_l.append(_f(inputs['eat_b']) @ ve)
        Wstk[:, (li - 1) * 4:(li - 1) * 4 + 4, :] = lw_h / HEADS
        gb[:, li - 1] = _f(inputs[f'g{li}_bias'])
    Vsrc = np.concatenate(Vs_l, 1)          # [128,8]
    Vdst = np.concatenate(Vd_l, 1)
    w_e4 = np.concatenate(w_e4_l, 1)        # [16,8]
    b4 = np.concatenate(b4_l, 0)            # [8]
    z16 = np.zeros((128, 16), np.float32)
    Vs_dup = np.ascontiguousarray(np.concatenate([Vsrc, Vsrc, z16], 1)).astype(ml_dtypes.bfloat16)
    Vd_dup = np.ascontiguousarray(np.concatenate([Vdst, Vdst, z16], 1)).astype(ml_dtypes.bfloat16)
    w18 = np.zeros((18, 32), np.float32)
    w18[:16, :16] = np.concatenate([w_e4, w_e4], 1)
    w18[16, :16] = np.concatenate([b4, b4])
    w18[17, :16] = -40.0
    w18 = w18.astype(ml_dtypes.bfloat16)

    src = ei[0].astype(np.int64)
    dst = ei[1].astype(np.int64)
    # host-computed self-loop attrs: mean of incoming raw edge_attr per node
    sums = np.zeros((N_NODES, 16), np.float32)
    np.add.at(sums, dst, ea)
    cnt_all = np.bincount(dst, minlength=N_NODES).astype(np.float32)
    loop_mean = sums / np.maximum(cnt_all, 1.0)[:, None]

    # ---- LSTM weights: permute gates to i,f,o,g; fold tanh->sigmoid (x2 on
    # g rows) and h = 2*hhat (x2 on h-input columns) ----
    w_ih0 = _f(inputs['w_ih0']).copy()
    w_hh0 = _f(inputs['w_hh0']).copy()
    b0 = (_f(inputs['b_ih0']) + _f(inputs['b_hh0'])).copy()
    w_ih1 = _f(inputs['w_ih1']).copy()
    w_hh1 = _f(inputs['w_hh1']).copy()
    b1 = (_f(inputs['b_ih1']) + _f(inputs['b_hh1'])).copy()
    for W, b in ((w_ih0, b0), (w_hh0, None), (w_ih1, b1), (w_hh1, None)):
        W[G_SLOT.start:G_SLOT.stop] *= 2.0
        if b is not None:
            b[G_SLOT.start:G_SLOT.stop] *= 2.0
    # x2 on recurrent-h input columns (h = 2*hhat)
    w_hh0 *= 2.0
    w_ih1 *= 2.0
    w_hh1 *= 2.0
    fcw2 = 2.0 * _f(inputs['fc_w']).reshape(128, 1)

    WseqT = np.zeros((SEQ_F + 1, 512), np.float32)
    WseqT[:SEQ_F] = w_ih0[:, :SEQ_F].T
    WseqT[SEQ_F] = b0
    shared = dict(
        Vs_dup=Vs_dup, Vd_dup=Vd_dup, w18=w18,
        Wstk=np.ascontiguousarray(Wstk.astype(ml_dtypes.bfloat16)), gb=gb,
        I128b=np.eye(128, dtype=ml_dtypes.bfloat16),
        iota16rep=np.ascontiguousarray(
            np.tile(np.arange(16, dtype=np.float32), (128, 8))).astype(ml_dtypes.bfloat16),
        ones_bf=np.ones((128, 1), ml_dtypes.bfloat16),
        WseqT=np.ascontiguousarray(WseqT).astype(ml_dtypes.bfloat16),
        Wg=np.ascontiguousarray(w_ih0[:, SEQ_F:].T).astype(ml_dtypes.bfloat16),
        Whh0=np.ascontiguousarray(w_hh0.T).astype(ml_dtypes.bfloat16),
        Wih1=np.ascontiguousarray(w_ih1.T).astype(ml_dtypes.bfloat16),
        Whh1=np.ascontiguousarray(w_hh1.T).astype(ml_dtypes.bfloat16),
        b1lhs=np.ascontiguousarray(b1.reshape(1, 4, 128)).astype(ml_dtypes.bfloat16),
        ones_row=np.ones((1, 512), ml_dtypes.bfloat16),
        fcw2=fcw2.astype(ml_dtypes.bfloat16),
        fcb=float(_f(inputs['fc_b']).reshape(-1)[0]),
    )

    cores = []
    for c in range(N_CORES):
        sel = nidx[c * BC:(c + 1) * BC].astype(np.int64)
        uniq = np.unique(sel)
        U = len(uniq)
        n_win = (U + 15) // 16
        U_pad = n_win * 16
        kd_pos = np.searchsorted(uniq, dst)
        keep = (kd_pos < U) & (uniq[np.minimum(kd_pos, U - 1)] == dst)
        ks = src[keep]
        ku = np.searchsorted(uniq, dst[keep])
        kea = ea[keep]
        order = np.argsort(ku, kind='stable')
        ks, ku, kea = ks[order], ku[order], kea[order]
        ubnd = np.searchsorted(ku, np.arange(0, U_pad + 16, 16))

        e_src, e_cdst, e_ea, e_kind = [], [], [], []
        win_off, win_nch = [], []
        for w in range(n_win):
            off = sum(len(a) for a in e_src)
            win_off.append(off)
            u0 = w * 16
            nreal_u = min(16, U - u0)
            ss = np.zeros(16, np.int64)
            ss[:nreal_u] = uniq[u0:u0 + nreal_u]
            e_src.append(ss)
            e_cdst.append(np.arange(16, dtype=np.float32))
            sea = np.zeros((16, 16), np.float32)
            sea[:nreal_u] = loop_mean[uniq[u0:u0 + nreal_u]]
            e_ea.append(sea)
            kk = np.full(16, 1, np.int64)
            kk[nreal_u:] = 2
            e_kind.append(kk)
            lo, hi = ubnd[w], ubnd[w + 1]
            nreal = hi - lo
            e_src.append(ks[lo:hi])
            e_cdst.append((ku[lo:hi] - u0).astype(np.float32))
            e_ea.append(kea[lo:hi])
            e_kind.append(np.zeros(nreal, np.int64))
            npad = (-(16 + nreal)) % 128
            if npad:
                e_src.append(np.zeros(npad, np.int64))
                e_cdst.append(np.zeros(npad, np.float32))
                e_ea.append(np.zeros((npad, 16), np.float32))
                e_kind.append(np.full(npad, 2, np.int64))
            win_nch.append((16 + nreal + npad) // 128)
        e_src = np.concatenate(e_src)
        e_cdst = np.concatenate(e_cdst)
        e_ea = np.concatenate(e_ea)
        e_kind = np.concatenate(e_kind)
        sq = seqs[c * BC:(c + 1) * BC]
        seqT = np.ones((SEQ_LEN, SEQ_F + 1, BC), np.float32)
        seqT[:, :SEQ_F, :] = sq.transpose(1, 2, 0)
        cores.append(dict(
            n_win=n_win, U=U, U_pad=U_pad, win_off=win_off, win_nch=win_nch,
            e_src=e_src, e_cdst=e_cdst, e_ea=e_ea, e_kind=e_kind,
            map_b=np.searchsorted(uniq, sel).astype(np.int64),
            seqT=np.ascontiguousarray(seqT).astype(ml_dtypes.bfloat16),
        ))

    # ---- uniform padding across cores: same n_win AND same chunks/window ----
    nwmax = max(co['n_win'] for co in cores)
    nchw = max(max(co['win_nch']) for co in cores)
    padW = nchw * 128
    for co in cores:
        ns, ncd, nea, nk = [], [], [], []
        new_off, new_nch = [], []
        for w in range(nwmax):
            new_off.append(w * padW)
            new_nch.append(nchw)
            if w < co['n_win']:
                off = co['win_off'][w]
                n = co['win_nch'][w] * 128
                ns.append(co['e_src'][off:off + n])
                ncd.append(co['e_cdst'][off:off + n])
                nea.append(co['e_ea'][off:off + n])
                nk.append(co['e_kind'][off:off + n])
                pad = padW - n
            else:
                pad = padW
            if pad:
                ns.append(np.zeros(pad, np.int64))
                ncd.append((np.arange(pad) % 16).astype(np.float32))
                nea.append(np.zeros((pad, 16), np.float32))
                nk.append(np.full(pad, 2, np.int64))
        co['e_src'] = np.concatenate(ns)
        co['e_cdst'] = np.concatenate(ncd)
        co['e_ea'] = np.concatenate(nea)
        co['e_kind'] = np.concatenate(nk)
        co['win_off'], co['win_nch'], co['n_win'] = new_off, new_nch, nwmax
    Emax = ((nwmax * padW + SPAN - 1) // SPAN) * SPAN
    for co in cores:
        add = Emax - len(co['e_src'])
        if add:
            co['e_src'] = np.concatenate([co['e_src'], np.zeros(add, np.int64)])
            co['e_cdst'] = np.concatenate([co['e_cdst'],
                                           (np.arange(add) % 16).astype(np.float32)])
            co['e_ea'] = np.concatenate([co['e_ea'], np.zeros((add, 16), np.float32)])
            co['e_kind'] = np.concatenate([co['e_kind'], np.full(add, 2, np.int64)])
        E, kind = Emax, co['e_kind']
        nch = E // 128
        eaT = np.zeros((18, E), np.float32)
        eaT[:16] = co['e_ea'].T
        eaT[16] = (kind != 2)
        eaT[17] = (kind == 2)
        # dst node ids for the d-gather
        dstn = np.zeros(E, np.int64)
        for w in range(co['n_win']):
            off = co['win_off'][w]
            n = co['win_nch'][w] * 128
            u_ids = co['e_src'][off:off + 16]
            j = np.minimum(co['e_cdst'][off:off + n].astype(np.int64), 15)
            dstn[off:off + n] = u_ids[j]
        s01 = (co['e_cdst'].reshape(nch, 128)[:, :, None] ==
               np.arange(16, dtype=np.float32)[None, None, :])
        co.update(
            E=E, nch=nch,
            eaT=np.ascontiguousarray(eaT).astype(ml_dtypes.bfloat16),
            Xg_h=np.ascontiguousarray(
                xb[co['e_src']].reshape(nch, 128, 128).transpose(1, 0, 2)),
            XsT_h=np.ascontiguousarray(xb[co['e_src']].T),
            XdT_h=np.ascontiguousarray(xb[dstn].T),
            s01=np.ascontiguousarray(
                s01.transpose(1, 0, 2).astype(np.float32)).astype(ml_dtypes.bfloat16),
            U_pad=nwmax * 16,
        )
        Sel = np.zeros((nwmax * 16, BC), np.float32)
        Sel[co['map_b'], np.arange(BC)] = 1.0
        co['Sel'] = np.ascontiguousarray(
            Sel.reshape(nwmax * 16 // 128, 128, BC).transpose(1, 0, 2)).astype(ml_dtypes.bfloat16)
    return cores, shared


def build_core_program(nc, co):
    E, nch, n_win, U_pad = co['E'], co['nch'], co['n_win'], co['U_pad']
    n_span = E // SPAN

    def din(name, shape, dt):
        return nc.dram_tensor(name, list(shape), dt, kind="ExternalInput")

    seqT_d = din('seqT', (SEQ_LEN, SEQ_F + 1, BC), BF16)
    Xg_d = din('Xg_h', (128, nch, 128), BF16)
    XsT_d = din('XsT_h', (128, E), BF16)
    XdT_d = din('XdT_h', (128, E), BF16)
    Sel_d = din('Sel', (128, U_pad // 128, BC), BF16)
    eaT_d = din('eaT', (18, E), BF16)
    s01_d = din('s01', (128, nch, 16), BF16)
    Vs_d = din('Vs_dup', (128, 32), BF16)
    Vd_d = din('Vd_dup', (128, 32), BF16)
    w18_d = din('w18', (18, 32), BF16)
    Wstk_d = din('Wstk', (128, 8, GAT_OUT), BF16)
    gb_d = din('gb', (GAT_OUT, 2), F32)
    I128b_d = din('I128b', (128, 128), BF16)
    iota_d = din('iota16rep', (128, 128), BF16)
    ones_d = din('ones_bf', (128, 1), BF16)
    WseqT_d = din('WseqT', (SEQ_F + 1, 512), BF16)
    Wg_d = din('Wg', (128, 512), BF16)
    Whh0_d = din('Whh0', (128, 512), BF16)
    Wih1_d = din('Wih1', (128, 512), BF16)
    Whh1_d = din('Whh1', (128, 512), BF16)
    b1lhs_d = din('b1lhs', (1, 4, 128), BF16)
    ones_row_d = din('ones_row', (1, 512), BF16)
    fcw2_d = din('fcw2', (128, 1), BF16)
    fcb_d = din('fcb', (1, 1), F32)
    y_d = nc.dram_tensor('y', [1, BC], F32, kind="ExternalOutput")

    # window/chunk bookkeeping (host-known)
    chunk_win = []
    for w in range(n_win):
        chunk_win += [w] * co['win_nch'][w]
    chunk_win += [-1] * (nch - len(chunk_win))
    win_first_last = {}
    for c, w in enumerate(chunk_win):
        if w < 0:
            continue
        if w not in win_first_last:
            win_first_last[w] = [c, c]
        win_first_last[w][1] = c
    WGRP = 4   # windows finalized per group

    import contextlib
    with tile.TileContext(nc) as tc:
        with contextlib.ExitStack() as ctx:
            consts = ctx.enter_context(tc.tile_pool(name="consts", bufs=1))

            def load(dram, shape, dt):
                nm = dram.ap().tensor.name
                t = consts.tile(list(shape), dt, name="c_" + nm, tag="c_" + nm)
                nc.sync.dma_start(t[:], dram.ap())
                return t

            I128b = load(I128b_d, (128, 128), BF16)
            iota = load(iota_d, (128, 128), BF16)
            ones = load(ones_d, (128, 1), BF16)
            Vs = load(Vs_d, (128, 32), BF16)
            Vd = load(Vd_d, (128, 32), BF16)
            w18 = load(w18_d, (18, 32), BF16)
            Wstk = load(Wstk_d, (128, 8, GAT_OUT), BF16)
            gbias = load(gb_d, (GAT_OUT, 2), F32)
            s01 = load(s01_d, (128, nch, 16), BF16)
            Sel = load(Sel_d, (128, U_pad // 128, BC), BF16)
            Wseq = load(WseqT_d, (SEQ_F + 1, 512), BF16)
            Wg = load(Wg_d, (128, 512), BF16)
            Whh0 = load(Whh0_d, (128, 512), BF16)
            Wih1 = load(Wih1_d, (128, 512), BF16)
            Whh1 = load(Whh1_d, (128, 512), BF16)
            b1lhs = load(b1lhs_d, (1, 4, 128), BF16)
            ones_row = load(ones_row_d, (1, 512), BF16)
            fcw2 = load(fcw2_d, (128, 1), BF16)
            fcb = load(fcb_d, (1, 1), F32)

            persist = ctx.enter_context(tc.tile_pool(name="persist", bufs=1))
            T_sb = persist.tile([128, n_span * 512], BF16)     # transposed exp(leaky) blocks
            AnT_all = persist.tile([128, n_win, 128], BF16)
            gcombT = persist.tile([128, BC], BF16)

            # ================= GAT =================
            with contextlib.ExitStack() as gctx:
                span_pool = gctx.enter_context(tc.tile_pool(name="span", bufs=3))
                sd_ps = gctx.enter_context(tc.tile_pool(name="sd_ps", bufs=2, space="PSUM"))
                g_pool = gctx.enter_context(tc.tile_pool(name="g", bufs=2))
                tp_ps = gctx.enter_context(tc.tile_pool(name="tp_ps", bufs=2, space="PSUM"))
                pall_pool = gctx.enter_context(tc.tile_pool(name="pall", bufs=4))
                agg_ps = gctx.enter_context(tc.tile_pool(name="agg_ps", bufs=2, space="PSUM"))
                den_ps = gctx.enter_context(tc.tile_pool(name="den_ps", bufs=2, space="PSUM"))
                fin_pool = gctx.enter_context(tc.tile_pool(name="fin", bufs=2))

                agg_group = {}
                for sp in range(n_span):
                    sc0 = sp * SPAN
                    eaT_sp = span_pool.tile([18, SPAN], BF16, tag="easp")
                    nc.sync.dma_start(eaT_sp[:], eaT_d.ap()[:, sc0:sc0 + SPAN])
                    Xg = span_pool.tile([128, SPAN // 128, 128], BF16, tag="xg")
                    nc.sync.dma_start(Xg[:], Xg_d.ap()[:, sc0 // 128:(sc0 + SPAN) // 128, :])
                    XsT = span_pool.tile([128, SPAN], BF16, tag="xst")
                    nc.sync.dma_start(XsT[:], XsT_d.ap()[:, sc0:sc0 + SPAN])
                    XdT = span_pool.tile([128, SPAN], BF16, tag="xdt")
                    nc.sync.dma_start(XdT[:], XdT_d.ap()[:, sc0:sc0 + SPAN])

                    # --- logits: S[32k+0:32k+32, :] for 4 k-chunks of 512 edges ---
                    S_ps = sd_ps.tile([128, 512], F32, tag="S")
                    for k in range(4):
                        cl = 512 * k
                        nc.tensor.matmul(S_ps[32 * k:32 * k + 32, :], lhsT=Vs[:],
                                         rhs=XsT[:, cl:cl + 512], start=True, stop=False,
                                         tile_position=(0, 32 * k))
                        nc.tensor.matmul(S_ps[32 * k:32 * k + 32, :], lhsT=Vd[:],
                                         rhs=XdT[:, cl:cl + 512], start=False, stop=False,
                                         tile_position=(0, 32 * k))
                        nc.tensor.matmul(S_ps[32 * k:32 * k + 32, :], lhsT=w18[:],
                                         rhs=eaT_sp[:, cl:cl + 512], start=False, stop=True,
                                         tile_position=(0, 32 * k))
                    # PSUM->SBUF copy on ACT (idle in GAT), then leaky on DVE
                    G1 = g_pool.tile([128, 512], BF16, tag="G1")
                    nc.scalar.copy(G1[:], S_ps[:])
                    G = g_pool.tile([128, 512], BF16, tag="G")
                    nc.vector.scalar_tensor_tensor(G[:], G1[:], 0.2, G1[:],
                                                   op0=ALU.mult, op1=ALU.max)
                    # transpose 4 blocks -> T region, then exp in-place
                    tps = tp_ps.tile([128, 512], BF16, tag="tps")
                    for jb in range(4):
                        nc.tensor.transpose(tps[:, 128 * jb:128 * jb + 128],
                                            G[:, 128 * jb:128 * jb + 128], I128b[:])
                    tcol0 = sp * 512
                    nc.scalar.activation(T_sb[:, tcol0:tcol0 + 512], tps[:], AF.Exp)

                    # --- pall for this span: 4 chunks per DVE op ---
                    # within-span chunk cc = 4k + jb lives at T cols 32k + 128jb;
                    # a group of 4 consecutive cc shares k, jb=0..3 (stride 128)
                    pall4s = []
                    for kq in range(4):
                        cbase = sc0 // 128 + 4 * kq
                        pall4 = pall_pool.tile([128, 4, 8, 16], BF16, tag="pall")
                        tview = (T_sb[:, sp * 512:(sp + 1) * 512]
                                 .rearrange("p (a b) -> p a b", a=4)
                                 [:, :, 32 * kq:32 * kq + 8]
                                 .unsqueeze(3).broadcast_to([128, 4, 8, 16]))
                        nc.vector.tensor_tensor(
                            pall4[:],
                            s01[:, cbase:cbase + 4, :].unsqueeze(2)
                            .broadcast_to([128, 4, 8, 16]),
                            tview, op=ALU.mult)
                        pall4s.append(pall4)

                    # --- aggregation for chunks in this span ---
                    for c in range(sc0 // 128, (sc0 + SPAN) // 128):
                        w = chunk_win[c]
                        if w < 0:
                            continue
                        grp = w // WGRP
                        wi = w % WGRP
                        c_first, c_last = win_first_last[w]
                        if wi == 0 and c == c_first:
                            agg_group[grp] = (
                                agg_ps.tile([128, WGRP, 128], F32, tag="aggp", name="aggp"),
                                den_ps.tile([128, WGRP], F32, tag="aggd", name="aggd"))
                        aggp, aggd = agg_group[grp]
                        cc = c - sc0 // 128
                        pall = pall4s[cc // 4][:, cc % 4, :, :].rearrange("p a b -> p (a b)")
                        nc.tensor.matmul(aggp[:, wi, :], lhsT=pall,
                                         rhs=Xg[:, cc, :],
                                         start=(c == c_first), stop=(c == c_last))
                        nc.tensor.matmul(aggd[:, wi:wi + 1], lhsT=pall, rhs=ones[:],
                                         start=(c == c_first), stop=(c == c_last))
                        if c == c_last and wi == WGRP - 1:
                            rec = fin_pool.tile([128, WGRP], F32, tag="rec")
                            nc.vector.reciprocal(rec[:], aggd[:])
                            anw = fin_pool.tile([128, WGRP, 128], BF16, tag="anw")
                            nc.vector.tensor_tensor(
                                anw[:], aggp[:],
                                rec[:].unsqueeze(2).broadcast_to([128, WGRP, 128]),
                                op=ALU.mult)
                            antp = tp_ps.tile([128, 512], BF16, tag="tps", name="antp")
                            for wj in range(WGRP):
                                nc.tensor.transpose(antp[:, 128 * wj:128 * wj + 128],
                                                    anw[:, wj, :], I128b[:])
                            w0 = grp * WGRP
                            nc.vector.tensor_copy(
                                AnT_all[:, w0:w0 + WGRP, :].rearrange("p a b -> p (a b)"),
                                antp[:])
                            del agg_group[grp]

                # --- projection + gcomb ---
                o1 = agg_ps.tile([GAT_OUT, U_pad], F32, tag="aggp", name="o1")
                o2 = agg_ps.tile([GAT_OUT, U_pad], F32, tag="aggp", name="o2")
                for h in range(4):
                    nc.tensor.matmul(o1[:], lhsT=Wstk[:, h, :],
                                     rhs=AnT_all[:, :, 16 * h:16 * h + 16],
                                     start=(h == 0), stop=(h == 3))
                    nc.tensor.matmul(o2[:], lhsT=Wstk[:, 4 + h, :],
                                     rhs=AnT_all[:, :, 64 + 16 * h:64 + 16 * h + 16],
                                     start=(h == 0), stop=(h == 3))
                gstk = fin_pool.tile([128, U_pad], BF16, tag="gstk")
                nc.scalar.add(gstk[0:64, :], o1[:], gbias[:, 0:1])
                nc.scalar.add(gstk[64:128, :], o2[:], gbias[:, 1:2])
                gsel = agg_ps.tile([128, BC], F32, tag="aggp", name="gsel")
                for uc in range(U_pad // 128):
                    gtp = tp_ps.tile([128, 128], BF16, tag="tps", name="gtp")
                    nc.tensor.transpose(gtp[:], gstk[:, 128 * uc:128 * uc + 128], I128b[:])
                    gts = fin_pool.tile([128, 128], BF16, tag="gts")
                    nc.vector.tensor_copy(gts[:], gtp[:])
                    nc.tensor.matmul(gsel[:], lhsT=gts[:], rhs=Sel[:, uc, :],
                                     start=(uc == 0), stop=(uc == U_pad // 128 - 1))
                nc.vector.tensor_copy(gcombT[:], gsel[:])

            # ================= LSTM (L1 lagged 2 steps behind L0) =================
            seq_pool = ctx.enter_context(tc.tile_pool(name="seq", bufs=2))
            ps0_pool = ctx.enter_context(tc.tile_pool(name="ps0", bufs=1, space="PSUM"))
            ps1_pool = ctx.enter_context(tc.tile_pool(name="ps1", bufs=1, space="PSUM"))
            st_pool = ctx.enter_context(tc.tile_pool(name="state", bufs=1))
            act_pool = ctx.enter_context(tc.tile_pool(name="acts", bufs=2))

            psum0 = ps0_pool.tile([128, 2048], F32)
            psum1 = ps1_pool.tile([128, 2048], F32)
            h0 = [st_pool.tile([128, BC], BF16, tag=f"h0{p}", name=f"h0{p}")
                  for p in (0, 1, 2)]
            h1 = [st_pool.tile([128, BC], BF16, tag=f"h1{p}", name=f"h1{p}")
                  for p in (0, 1)]
            ct0 = [st_pool.tile([128, BC], FP16, tag=f"c0{p}", name=f"c0{p}")
                   for p in (0, 1)]
            ct1 = [st_pool.tile([128, BC], FP16, tag=f"c1{p}", name=f"c1{p}")
                   for p in (0, 1)]
            for t_ in h0 + h1 + ct0 + ct1:
                nc.vector.memset(t_[:], 0.0)

            def l0_mm_in(t, seqb):
                # seq + gcomb matmuls for step t (group start; whh closes it)
                for g in range(4):
                    o = psum0[:, 512 * g:512 * g + 512]
                    nc.tensor.matmul(o, lhsT=Wseq[:, 128 * g:128 * g + 128],
                                     rhs=seqb[:, t % TBLK, :], start=True, stop=False)
                    nc.tensor.matmul(o, lhsT=Wg[:, 128 * g:128 * g + 128],
                                     rhs=gcombT[:], start=False, stop=False)

            def l0_mm_rec(t):
                for g in range(4):
                    nc.tensor.matmul(psum0[:, 512 * g:512 * g + 512],
                                     lhsT=Whh0[:, 128 * g:128 * g + 128],
                                     rhs=h0[t % 3][:], start=False, stop=True)

            TBLK = 10

            def seq_block(t):
                sq = seq_pool.tile([SEQ_F + 1, TBLK, BC], BF16, name="seqb")
                nc.sync.dma_start(
                    sq[:], seqT_d.ap()[t:t + TBLK, :, :].rearrange("t p b -> p t b"))
                return sq

            seqb = None
            for t in range(SEQ_LEN + 2):
                if t == 0:
                    seqb = seq_block(0)
                    l0_mm_in(0, seqb)
                    l0_mm_rec(-1)   # h0 init (zeros) lives in slot 2
                # ---- L1 matmuls for time t-2 (fully off the L0 chain) ----
                if t >= 2:
                    sig1 = act_pool.tile([128, 2048], FP16, tag="sig1")
                    for g in range(4):
                        o = psum1[:, 512 * g:512 * g + 512]
                        nc.tensor.matmul(o, lhsT=b1lhs[:, g, :], rhs=ones_row[:],
                                         start=True, stop=False)
                        nc.tensor.matmul(o, lhsT=Wih1[:, 128 * g:128 * g + 128],
                                         rhs=h0[(t - 2) % 3][:], start=False, stop=False)
                        nc.tensor.matmul(o, lhsT=Whh1[:, 128 * g:128 * g + 128],
                                         rhs=h1[(t - 1) % 2][:], start=False, stop=True)
                if t < SEQ_LEN:
                    # ---- L0 chain: sigmoid(i,f,g), sigma(o), c0, sigma(c0), h0 ----
                    sig0a = act_pool.tile([128, 1536], FP16, tag="sig0a")
                    nc.scalar.activation(sig0a[:], psum0[:, 0:1536], AF.Sigmoid)
                    sig0b = act_pool.tile([128, BC], FP16, tag="sig0b")
                    nc.scalar.activation(sig0b[:], psum0[:, 1536:2048], AF.Sigmoid)
                    if t + 1 < SEQ_LEN:
                        if (t + 1) % TBLK == 0:
                            seqb = seq_block(t + 1)
                        l0_mm_in(t + 1, seqb)
                    t1_0 = act_pool.tile([128, BC], FP16, tag="t10")
                    nc.vector.scalar_tensor_tensor(t1_0[:], sig0a[:, 1024:1536], -0.5,
                                                   sig0a[:, 0:512],
                                                   op0=ALU.add, op1=ALU.mult)
                    t2_0 = act_pool.tile([128, BC], FP16, tag="t20")
                    nc.vector.tensor_tensor(t2_0[:], sig0a[:, 512:1024],
                                            ct0[(t - 1) % 2][:], op=ALU.mult)
                    nc.vector.scalar_tensor_tensor(ct0[t % 2][:], t1_0[:], 4.0,
                                                   t2_0[:], op0=ALU.mult, op1=ALU.add)
                    sc0 = act_pool.tile([128, BC], FP16, tag="sc0")
                    nc.scalar.activation(sc0[:], ct0[t % 2][:], AF.Sigmoid)
                    nc.vector.scalar_tensor_tensor(h0[t % 3][:], sc0[:], -0.5,
                                                   sig0b[:], op0=ALU.add, op1=ALU.mult)
                    if t + 1 < SEQ_LEN:
                        l0_mm_rec(t)
                # ---- L1 tail for time t-2 ----
                if t >= 2:
                    nc.scalar.activation(sig1[:], psum1[:], AF.Sigmoid)
                    t1_1 = act_pool.tile([128, BC], FP16, tag="t11")
                    nc.vector.scalar_tensor_tensor(t1_1[:], sig1[:, 1024:1536], -0.5,
                                                   sig1[:, 0:512],
                                                   op0=ALU.add, op1=ALU.mult)
                    t2_1 = act_pool.tile([128, BC], FP16, tag="t21")
                    nc.gpsimd.tensor_tensor(t2_1[:], sig1[:, 512:1024],
                                            ct1[(t - 1) % 2][:], op=ALU.mult)
                    nc.vector.scalar_tensor_tensor(ct1[t % 2][:], t1_1[:], 4.0,
                                                   t2_1[:], op0=ALU.mult, op1=ALU.add)
                    sc1 = act_pool.tile([128, BC], FP16, tag="sc1")
                    nc.scalar.activation(sc1[:], ct1[t % 2][:], AF.Sigmoid)
                    nc.vector.scalar_tensor_tensor(h1[t % 2][:], sc1[:], -0.5,
                                                   sig1[:, 1536:2048],
                                                   op0=ALU.add, op1=ALU.mult)

            # ---------------- fc ----------------
            yps = psum0[0:1, 0:BC]
            nc.tensor.matmul(yps, lhsT=fcw2[:], rhs=h1[(SEQ_LEN + 1) % 2][:],
                             start=True, stop=True)
            ysb = act_pool.tile([1, BC], F32, tag="ysb")
            nc.scalar.add(ysb[:], yps, fcb[:1, :1])
            nc.sync.dma_start(y_d.ap(), ysb[:])


def kernel(**inputs):
    cores, sh = host_prep(inputs)
    co0 = cores[0]

    nc = bacc.Bacc("TRN2", target_bir_lowering=False, debug=False, num_devices=1)
    build_core_program(nc, co0)
    nc.compile()

    in_maps = []
    for co in cores:
        m = dict(
            seqT=co['seqT'], Xg_h=co['Xg_h'], XsT_h=co['XsT_h'], XdT_h=co['XdT_h'],
            Sel=co['Sel'], eaT=co['eaT'], s01=co['s01'],
            fcb=np.array([[sh['fcb']]], np.float32),
        )
        for k in ('Vs_dup', 'Vd_dup', 'w18', 'Wstk', 'gb', 'I128b', 'iota16rep',
                  'ones_bf', 'WseqT', 'Wg', 'Whh0', 'Wih1', 'Whh1', 'b1lhs',
                  'ones_row', 'fcw2'):
            m[k] = sh[k]
        in_maps.append(m)

    if os.environ.get("BK_SIM"):
        from concourse.bass_interp import CoreSim
        ncore = int(os.environ.get("BK_SIM_CORES", "1"))
        outs = []
        for ci in range(ncore):
            sim = CoreSim(nc, require_finite=False, require_nnan=False)
            for k, v in in_maps[ci].items():
                sim.tensor(k)[:] = v
            sim.simulate(check_with_hw=False)
            outs.append(np.array(sim.tensor('y')).reshape(BC, 1).copy())
        for ci in range(ncore, N_CORES):
            outs.append(np.zeros((BC, 1), np.float32))
        return np.concatenate(outs, 0)

    trace = bool(os.environ.get("BK_TRACE"))
    res = bass_utils.run_bass_kernel_spmd(nc, in_maps, core_ids=list(range(N_CORES)),
                                          trace=trace)
    if trace:
        global LAST_EXEC_NS, LAST_RES
        LAST_EXEC_NS = res.exec_time_ns
        LAST_RES = res
        print("HW exec time:", res.exec_time_ns, "ns")
    return np.concatenate([res.results[c]['y'].reshape(BC, 1) for c in range(N_CORES)], 0)


LAST_EXEC_NS = None
LAST_RES = None
